# revision 13
# baseline (speedup 1.0000x reference)
"""Trainium2 Bass kernel for a 4-layer dense transformer (B=2, S=2048, D=1024,
H=16, F=4096, V=32000) running on 8 NeuronCores.

Sharding: 2-way data parallel over batch x 4-way sequence sharding within each
batch element (512 tokens per core). Layer weights are replicated per core and
streamed from HBM; attention does a per-layer K/V all-gather within each
4-core batch group. The lm_head is sharded over vocab (4000 cols per core)
after an 8-rank all-gather of the final hidden states.

Activations are kept transposed on-chip: hT[d, t] with d on partitions, so all
projections consume weight tiles as lhsT directly and the token count (512) is
the matmul free dim. Softmax runs without max-subtraction (scores are O(1) for
this model); denominators come from a ones-column appended to V (via its bias
term), so the softmax sum falls out of the ctx matmul for free.
"""

import numpy as np

L, D, H, F, V = 4, 1024, 16, 4096, 32000
B, S = 2, 2048
HD = D // H          # 64
NCORES = 8
TL = 512             # tokens per core
P = 128
DI = D // P          # 8 d-chunks
FC = F // P          # 32 f-chunks
KC = S // P          # 16 key chunks per batch
VSH = V // NCORES    # 4000 vocab cols per core
VPAD = 4096
NV = VPAD // 512     # 8 n-chunks of lm head per core
EPS = 1e-5
SCALE = 1.0 / np.sqrt(HD)
VA = 65              # per-head v width incl. ones column
VCH = [(0, 7), (7, 7), (14, 2)]   # head-aligned v column chunks (h0, nheads)
GROUPS4 = [[0, 1, 2, 3], [4, 5, 6, 7]]
GROUPS8 = [list(range(8))]

_CACHE = {}


def _pack_weights(embed, pe, wq, bq, wk, bk, wv, bv, wo, bo,
                  ln1_g, ln1_b, w1, b1, w2, b2, ln2_g, ln2_b, lnf_g, lnf_b,
                  lm_head, input_ids):
    """Host-side packing of weights into tile layouts.
    Returns (shared inputs, per-core inputs, flags)."""
    f32 = np.float32
    asf = lambda x: np.asarray(x, dtype=f32)
    wq, bq, wk, bk, wv, bv, wo, bo = map(asf, (wq, bq, wk, bk, wv, bv, wo, bo))
    w1, b1, w2, b2 = map(asf, (w1, b1, w2, b2))
    ln1_g, ln1_b, ln2_g, ln2_b = map(asf, (ln1_g, ln1_b, ln2_g, ln2_b))
    lnf_g, lnf_b = asf(lnf_g), asf(lnf_b)

    def lhst_tiles(w):
        # w [K, M] -> [M/P, P(k), K/P, P(m)]: row-block layout matching the
        # SBUF tile [P, K/P, P] so the DMA is a plain contiguous copy
        Kd, M = w.shape
        t = w.reshape(Kd // P, P, M // P, P)          # ki, pk, mi, pm
        return np.ascontiguousarray(t.transpose(2, 1, 0, 3))

    shared = {}
    shared['wqp'] = np.stack([lhst_tiles(wq[l]) for l in range(L)])
    shared['wkp'] = np.stack([lhst_tiles(wk[l]) for l in range(L)])
    shared['wop'] = np.stack([lhst_tiles(wo[l]) for l in range(L)])
    shared['w1p'] = np.stack([lhst_tiles(w1[l]) for l in range(L)])
    shared['w2p'] = np.stack([lhst_tiles(w2[l]) for l in range(L)])

    # v augmented: per head 64 cols of wv + 1 zero col; bias gets 1.0 there
    wv_aug = np.zeros((L, D, H * VA), dtype=f32)
    bv_aug = np.zeros((L, H * VA), dtype=f32)
    for h in range(H):
        wv_aug[:, :, h * VA:h * VA + HD] = wv[:, :, h * HD:(h + 1) * HD]
        bv_aug[:, h * VA:h * VA + HD] = bv[:, h * HD:(h + 1) * HD]
        bv_aug[:, h * VA + HD] = 1.0
    shared['wvp'] = np.ascontiguousarray(wv_aug.reshape(L, DI, P, H * VA))
    shared['bvp'] = bv_aug.reshape(L, 1, H * VA)

    # per-partition biases packed [L, P, nch]
    pk = lambda b, n: np.ascontiguousarray(
        b.reshape(L, n, P).transpose(0, 2, 1))
    shared['bqp'] = pk(bq, DI)
    shared['bkp'] = pk(bk, DI)
    shared['b1p'] = pk(b1, FC)
    shared['bop'] = pk(bo, DI)
    shared['b2p'] = b2.reshape(L, 1, DI, P)   # lhsT [1, P] tiles for bias mm

    # LN params: K=2 lhsT tiles [nln, di, 2(b,g), P] + per-partition g
    lnp = np.zeros((2 * L + 1, DI, 2, P), dtype=f32)
    lng = np.zeros((2 * L + 1, P, DI), dtype=f32)
    for l in range(L):
        for j, (g, b) in enumerate(((ln1_g, ln1_b), (ln2_g, ln2_b))):
            lnp[2 * l + j, :, 0, :] = b[l].reshape(DI, P)
            lnp[2 * l + j, :, 1, :] = g[l].reshape(DI, P)
            lng[2 * l + j] = g[l].reshape(DI, P).T
    lnp[2 * L, :, 0, :] = lnf_b.reshape(DI, P)
    lnp[2 * L, :, 1, :] = lnf_g.reshape(DI, P)
    lng[2 * L] = lnf_g.reshape(DI, P).T
    shared['lnp'] = lnp
    shared['lng'] = lng

    flags = {'bo': bool(np.any(bo)), 'b2': bool(np.any(b2))}

    # per-core: tokT, peT, lm_head shard
    per_core = []
    ids = np.asarray(input_ids).astype(np.int64)
    emb = asf(embed)
    pe_np = asf(pe)
    lmh = asf(lm_head)
    for r in range(NCORES):
        b_, q_ = r // 4, r % 4
        tok = emb[ids[b_, q_ * TL:(q_ + 1) * TL]]            # [TL, D]
        tokT = np.ascontiguousarray(tok.T).reshape(DI, P, TL)
        peT = np.ascontiguousarray(
            pe_np[q_ * TL:(q_ + 1) * TL, :].T).reshape(DI, P, TL)
        slp = np.zeros((D, VPAD), dtype=f32)
        slp[:, :VSH] = lmh[:, r * VSH:(r + 1) * VSH]
        lmt = np.ascontiguousarray(
            slp.reshape(DI, P, NV, 512).transpose(2, 1, 0, 3))
        per_core.append({'tokT': tokT, 'peT': peT, 'lmhp': lmt})
    return shared, per_core, flags


def _build(flags):
    import concourse.bass as bass  # noqa: F401
    import concourse.tile as tile
    from concourse import bacc, mybir
    from contextlib import ExitStack

    dt = mybir.dt.float32
    AF = mybir.ActivationFunctionType
    OP = mybir.AluOpType

    nc = bacc.Bacc("TRN2", target_bir_lowering=False, debug=False,
                   num_devices=NCORES)

    def din(name, shape):
        return nc.dram_tensor(name, list(shape), dt, kind="ExternalInput").ap()

    tokT_d = din('tokT', (DI, P, TL))
    peT_d = din('peT', (DI, P, TL))
    wqp = din('wqp', (L, DI, P, DI, P))
    wkp = din('wkp', (L, DI, P, DI, P))
    wop = din('wop', (L, DI, P, DI, P))
    w1p = din('w1p', (L, FC, P, DI, P))
    w2p = din('w2p', (L, DI, P, FC, P))
    wvp = din('wvp', (L, DI, P, H * VA))
    bvp = din('bvp', (L, 1, H * VA))
    bqp = din('bqp', (L, P, DI))
    bkp = din('bkp', (L, P, DI))
    b1p = din('b1p', (L, P, FC))
    bop = din('bop', (L, P, DI))
    b2p = din('b2p', (L, 1, DI, P))
    lnp_d = din('lnp', (2 * L + 1, DI, 2, P))
    lng_d = din('lng', (2 * L + 1, P, DI))
    lmhp = din('lmhp', (NV, P, DI, 512))
    logits_d = nc.dram_tensor('logits', [NCORES * TL, VPAD], dt,
                              kind="ExternalOutput").ap()

    with tile.TileContext(nc) as tc, ExitStack() as top:
        const = top.enter_context(tc.tile_pool(name="const", bufs=1))
        ones_col = const.tile([P, 1], dt)      # lhsT for partition-sum MMs
        nc.vector.memset(ones_col, 1.0)
        ones_row = const.tile([1, P], dt)      # lhsT for K=1 broadcast MMs
        nc.vector.memset(ones_row, 1.0)
        ones512 = const.tile([1, TL], dt)      # rhs for per-partition bias MMs
        nc.vector.memset(ones512, 1.0)
        eps_t = const.tile([1, 1], dt)
        nc.vector.memset(eps_t, EPS)

        hpool = top.enter_context(tc.tile_pool(name="h", bufs=1))
        hT = hpool.tile([P, DI, TL], dt, tag="hT")

        # one psum pool; tags: mm(2) + sc(2) + ctx(2) + small(2) = 8 banks
        pp = top.enter_context(tc.tile_pool(name="pp", bufs=2, space="PSUM"))
        dram = top.enter_context(tc.tile_pool(name="dram", bufs=2,
                                              space="DRAM"))

        def layernorm(x, ln_idx, scr, biasp):
            """In-place layernorm over d (partitions x chunks) of x."""
            ps_mean = pp.tile([1, TL], dt, tag="small")
            for dc in range(DI):
                nc.tensor.matmul(ps_mean, ones_col, x[:, dc, :],
                                 start=(dc == 0), stop=(dc == DI - 1))
            ps_sq = pp.tile([1, TL], dt, tag="small")
            for dc in range(DI):
                sq = scr.tile([P, TL], dt, tag="lnt")
                nc.scalar.activation(sq, x[:, dc, :], AF.Square)
                nc.tensor.matmul(ps_sq, ones_col, sq,
                                 start=(dc == 0), stop=(dc == DI - 1))
            m = scr.tile([1, TL], dt, tag="m", bufs=1)       # -mean
            nc.scalar.activation(m, ps_mean, AF.Copy, scale=-1.0 / D)
            var = scr.tile([1, TL], dt, tag="var", bufs=1)
            nc.vector.tensor_tensor(var, m, m, OP.mult)          # m^2
            msq = scr.tile([1, TL], dt, tag="msq", bufs=1)
            nc.scalar.activation(msq, ps_sq, AF.Copy, scale=1.0 / D)
            nc.vector.tensor_tensor(var, msq, var, OP.subtract)  # variance
            nc.scalar.activation(var, var, AF.Sqrt, bias=eps_t)  # stddev
            s = scr.tile([1, TL], dt, tag="s", bufs=1)
            nc.vector.reciprocal(s, var)                         # rstd
            u = scr.tile([1, TL], dt, tag="u", bufs=1)           # -m*s
            nc.vector.tensor_tensor(u, m, s, OP.mult)
            ps_s = pp.tile([P, TL], dt, tag="sc")      # s bcast over parts
            nc.tensor.matmul(ps_s, ones_row, s, start=True, stop=True)
            lgt = biasp.tile([P, DI], dt, tag="lng")
            nc.sync.dma_start(lgt, lng_d[ln_idx])
            for dc in range(DI):
                lnb = biasp.tile([1, P], dt, tag="lnb", bufs=2)
                nc.sync.dma_start(lnb, lnp_d[ln_idx, dc, 0:1, :])
                lngr = biasp.tile([1, P], dt, tag="lngr", bufs=2)
                nc.sync.dma_start(lngr, lnp_d[ln_idx, dc, 1:2, :])
                ps_b = pp.tile([P, TL], dt, tag="small")
                nc.tensor.matmul(ps_b, lnb, ones512, start=True, stop=False)
                nc.tensor.matmul(ps_b, lngr, u, start=False, stop=True)
                t = scr.tile([P, TL], dt, tag="lnt")
                nc.vector.tensor_tensor(t, x[:, dc, :], ps_s, OP.mult)
                nc.vector.scalar_tensor_tensor(
                    x[:, dc, :], t, lgt[:, dc:dc + 1], ps_b, OP.mult, OP.add)

        with ExitStack() as body:
            wrow = body.enter_context(tc.tile_pool(name="wrow", bufs=3))
            w2row = body.enter_context(tc.tile_pool(name="w2row", bufs=2))
            wvsl = body.enter_context(tc.tile_pool(name="wvsl", bufs=8))
            qpool = body.enter_context(tc.tile_pool(name="q", bufs=1))
            cpool = body.enter_context(tc.tile_pool(name="ctx", bufs=1))
            kpool = body.enter_context(tc.tile_pool(name="khp", bufs=2))
            epool = body.enter_context(tc.tile_pool(name="expp", bufs=2))
            vpool = body.enter_context(tc.tile_pool(name="vt", bufs=4))
            gpool = body.enter_context(tc.tile_pool(name="g", bufs=1))
            scr = body.enter_context(tc.tile_pool(name="scr", bufs=2))
            biasp = body.enter_context(tc.tile_pool(name="bias", bufs=2))

            # h0 = 2*tok + pe (transposed layout)
            for dc in range(DI):
                tk = scr.tile([P, TL], dt, tag="kd")
                nc.sync.dma_start(tk, tokT_d[dc])
                pec = scr.tile([P, TL], dt, tag="lnt")
                nc.sync.dma_start(pec, peT_d[dc])
                nc.vector.tensor_scalar(hT[:, dc, :], tk, 2.0, None, OP.mult)
                nc.vector.tensor_tensor(hT[:, dc, :], hT[:, dc, :], pec,
                                        OP.add)

            for l in range(L):
                # ---- K projection -> DRAM -> AllGather ----
                kT_loc = dram.tile([DI, P, TL], dt, tag="kloc")
                bk_sb = biasp.tile([P, DI], dt, tag="bk")
                nc.sync.dma_start(bk_sb, bkp[l])
                for dc in range(DI):
                    wr = wrow.tile([P, DI, P], dt, tag="wr")
                    nc.sync.dma_start(wr, wkp[l, dc])
                    ps = pp.tile([P, TL], dt, tag="mm")
                    for di in range(DI):
                        nc.tensor.matmul(ps, wr[:, di, :], hT[:, di, :],
                                         start=(di == 0), stop=(di == DI - 1))
                    kd = scr.tile([P, TL], dt, tag="kd")
                    nc.scalar.activation(kd, ps, AF.Identity,
                                         bias=bk_sb[:, dc:dc + 1])
                    nc.sync.dma_start(kT_loc[dc], kd)
                kT_full = dram.tile([4, DI, P, TL], dt, tag="kfull")
                nc.gpsimd.collective_compute(
                    "AllGather", OP.bypass,
                    ins=[kT_loc.opt()], outs=[kT_full.opt()],
                    replica_groups=GROUPS4)

                # ---- V projection (token-major, augmented cols) ----
                v_loc = dram.tile([H, TL, VA], dt, tag="vloc")
                for (h0, nh) in VCH:
                    c0, cw = h0 * VA, nh * VA
                    bv_sb = biasp.tile([1, 7 * VA], dt, tag="bv", bufs=2)
                    nc.sync.dma_start(bv_sb[:, :cw], bvp[l][:, c0:c0 + cw])
                    wv_tiles = []
                    for di in range(DI):
                        wt = wvsl.tile([P, 7 * VA], dt, tag="wv")
                        nc.sync.dma_start(wt[:, :cw], wvp[l, di, :, c0:c0 + cw])
                        wv_tiles.append(wt)
                    for t4 in range(4):
                        ps = pp.tile([P, 7 * VA], dt, tag="mm")
                        for di in range(DI):
                            nc.tensor.matmul(
                                ps[:, :cw],
                                hT[:, di, t4 * P:(t4 + 1) * P],
                                wv_tiles[di][:, :cw],
                                start=(di == 0), stop=False)
                        nc.tensor.matmul(ps[:, :cw], ones_row,
                                         bv_sb[:, :cw],
                                         start=False, stop=True)
                        vd = scr.tile([P, TL], dt, tag="kd")
                        nc.scalar.activation(vd[:, :cw], ps[:, :cw], AF.Copy)
                        for j in range(nh):
                            nc.sync.dma_start(
                                v_loc[h0 + j, t4 * P:(t4 + 1) * P, :],
                                vd[:, j * VA:(j + 1) * VA])
                v_full = dram.tile([4, H, TL, VA], dt, tag="vfull")
                nc.gpsimd.collective_compute(
                    "AllGather", OP.bypass,
                    ins=[v_loc.opt()], outs=[v_full.opt()],
                    replica_groups=GROUPS4)

                # ---- Q projection (stays in SBUF) ----
                qT = qpool.tile([P, DI, TL], dt, tag="qT")
                bq_sb = biasp.tile([P, DI], dt, tag="bq")
                nc.sync.dma_start(bq_sb, bqp[l])
                for dc in range(DI):
                    wr = wrow.tile([P, DI, P], dt, tag="wr")
                    nc.sync.dma_start(wr, wqp[l, dc])
                    ps = pp.tile([P, TL], dt, tag="mm")
                    for di in range(DI):
                        nc.tensor.matmul(ps, wr[:, di, :], hT[:, di, :],
                                         start=(di == 0), stop=(di == DI - 1))
                    nc.scalar.activation(qT[:, dc, :], ps, AF.Identity,
                                         bias=bq_sb[:, dc:dc + 1])

                # ---- attention ----
                ctxT = cpool.tile([P, DI, TL], dt, tag="ctxT")
                for hp in range(H // 2):
                    khp = kpool.tile([P, 4, TL], dt, tag="khp")
                    for r in range(4):
                        nc.sync.dma_start(khp[:, r, :], kT_full[r, hp])
                    for hh in range(2):
                        h = hp * 2 + hh
                        pb = hh * HD
                        ps_ctx = pp.tile([VA, TL], dt, tag="ctx")
                        for kc in range(KC):
                            ps_sc = pp.tile([P, TL], dt, tag="sc")
                            nc.tensor.matmul(
                                ps_sc,
                                khp[pb:pb + HD, kc // 4,
                                    (kc % 4) * P:(kc % 4 + 1) * P],
                                qT[pb:pb + HD, hp, :],
                                start=True, stop=True)
                            ex = epool.tile([P, TL], dt, tag="ex")
                            nc.scalar.activation(ex, ps_sc, AF.Exp,
                                                 scale=SCALE)
                            vt = vpool.tile([P, VA], dt, tag="vt")
                            nc.sync.dma_start(
                                vt,
                                v_full[kc // 4, h,
                                       (kc % 4) * P:(kc % 4 + 1) * P, :])
                            nc.tensor.matmul(ps_ctx, vt, ex,
                                             start=(kc == 0),
                                             stop=(kc == KC - 1))
                        rec = scr.tile([1, TL], dt, tag="rec", bufs=2)
                        nc.vector.reciprocal(rec, ps_ctx[HD:HD + 1, :])
                        ps_bc = pp.tile([HD, TL], dt, tag="small")
                        nc.tensor.matmul(ps_bc, ones_row[:, :HD], rec,
                                         start=True, stop=True)
                        nc.scalar.activation(ctxT[pb:pb + HD, hp, :],
                                             ps_ctx[0:HD, :], AF.Copy)
                        nc.vector.tensor_tensor(
                            ctxT[pb:pb + HD, hp, :],
                            ctxT[pb:pb + HD, hp, :], ps_bc, OP.mult)

                # ---- O projection + residual ----
                bo_sb = None
                if flags['bo']:
                    bo_sb = biasp.tile([P, DI], dt, tag="bo")
                    nc.sync.dma_start(bo_sb, bop[l])
                for dc in range(DI):
                    wr = wrow.tile([P, DI, P], dt, tag="wr")
                    nc.sync.dma_start(wr, wop[l, dc])
                    ps = pp.tile([P, TL], dt, tag="mm")
                    for di in range(DI):
                        nc.tensor.matmul(ps, wr[:, di, :], ctxT[:, di, :],
                                         start=(di == 0), stop=(di == DI - 1))
                    if bo_sb is not None:
                        nc.vector.scalar_tensor_tensor(
                            hT[:, dc, :], ps, bo_sb[:, dc:dc + 1],
                            hT[:, dc, :], OP.add, OP.add)
                    else:
                        nc.vector.tensor_tensor(hT[:, dc, :], ps,
                                                hT[:, dc, :], OP.add)
                layernorm(hT, 2 * l, scr, biasp)

                # ---- FFN ----
                gT = gpool.tile([P, FC, TL], dt, tag="gT")
                b1_sb = biasp.tile([P, FC], dt, tag="b1")
                nc.sync.dma_start(b1_sb, b1p[l])
                for fc in range(FC):
                    wr = wrow.tile([P, DI, P], dt, tag="wr")
                    nc.sync.dma_start(wr, w1p[l, fc])
                    ps = pp.tile([P, TL], dt, tag="mm")
                    for di in range(DI):
                        nc.tensor.matmul(ps, wr[:, di, :], hT[:, di, :],
                                         start=(di == 0), stop=(di == DI - 1))
                    nc.scalar.activation(gT[:, fc, :], ps, AF.Gelu,
                                         bias=b1_sb[:, fc:fc + 1])
                b2_sb = None
                if flags['b2']:
                    b2_sb = biasp.tile([1, DI, P], dt, tag="b2")
                    nc.sync.dma_start(b2_sb, b2p[l])
                for dc in range(DI):
                    ps = pp.tile([P, TL], dt, tag="sc")
                    for half in range(2):
                        w2r = w2row.tile([P, FC // 2, P], dt, tag="w2r")
                        nc.sync.dma_start(
                            w2r,
                            w2p[l, dc][:, half * FC // 2:(half + 1) * FC // 2,
                                       :])
                        for j in range(FC // 2):
                            fc = half * FC // 2 + j
                            nc.tensor.matmul(
                                ps, w2r[:, j, :], gT[:, fc, :],
                                start=(fc == 0),
                                stop=(fc == FC - 1 and b2_sb is None))
                    if b2_sb is not None:
                        nc.tensor.matmul(ps, b2_sb[:, dc, :], ones512,
                                         start=False, stop=True)
                    nc.vector.tensor_tensor(hT[:, dc, :], ps, hT[:, dc, :],
                                            OP.add)
                layernorm(hT, 2 * l + 1, scr, biasp)

            # final LN + store h for all-gather
            layernorm(hT, 2 * L, scr, biasp)
            hf_loc = dram.tile([DI, P, TL], dt, tag="hfloc")
            for dc in range(DI):
                nc.sync.dma_start(hf_loc[dc], hT[:, dc, :])
            hf_full = dram.tile([NCORES, DI, P, TL], dt, tag="hffull")
            nc.gpsimd.collective_compute(
                "AllGather", OP.bypass,
                ins=[hf_loc.opt()], outs=[hf_full.opt()],
                replica_groups=GROUPS8)

        # ---- lm head (vocab shard) ----
        with ExitStack() as lmctx:
            hfp = lmctx.enter_context(tc.tile_pool(name="hf", bufs=1))
            lmp = lmctx.enter_context(tc.tile_pool(name="lmw", bufs=2))
            outp = lmctx.enter_context(tc.tile_pool(name="lout", bufs=4))
            hf_sb = hfp.tile([P, DI, NCORES, TL], dt, tag="hf")
            for r in range(NCORES):
                nc.sync.dma_start(hf_sb[:, :, r, :],
                                  hf_full[r].rearrange("d p t -> p d t"))
            for vc in range(NV):
                lmr = lmp.tile([P, DI, 512], dt, tag="lmr")
                nc.sync.dma_start(lmr, lmhp[vc])
                for tcn in range(NCORES * TL // P):
                    r, lt = tcn // 4, tcn % 4
                    ps = pp.tile([P, 512], dt, tag="mm")
                    for di in range(DI):
                        nc.tensor.matmul(
                            ps,
                            hf_sb[:, di, r, lt * P:(lt + 1) * P],
                            lmr[:, di, :],
                            start=(di == 0), stop=(di == DI - 1))
                    ot = outp.tile([P, 512], dt, tag="ot")
                    nc.scalar.activation(ot, ps, AF.Copy)
                    nc.sync.dma_start(
                        logits_d[tcn * P:(tcn + 1) * P,
                                 vc * 512:(vc + 1) * 512], ot)

    nc.compile()
    return nc


def kernel(**inputs):
    from concourse.bass_utils import run_bass_kernel_spmd

    shared, per_core, flags = _pack_weights(**inputs)
    key = tuple(sorted(flags.items()))
    if key not in _CACHE:
        _CACHE[key] = _build(flags)
    nc = _CACHE[key]

    in_maps = []
    for r in range(NCORES):
        m = dict(shared)
        m.update(per_core[r])
        in_maps.append(m)
    res = run_bass_kernel_spmd(nc, in_maps, list(range(NCORES)))
    logits = np.zeros((B, S, V), dtype=np.float32)
    flat = logits.reshape(B * S, V)
    for r in range(NCORES):
        flat[:, r * VSH:(r + 1) * VSH] = res.results[r]['logits'][:, :VSH]
    return logits


# revision 15
# speedup vs baseline: 2.3776x; 2.3776x over previous
"""Trainium2 Bass kernel for a 4-layer dense transformer (B=2, S=2048, D=1024,
H=16, F=4096, V=32000) running on 8 NeuronCores.

Sharding: 2-way data parallel over batch x 4-way sequence sharding within each
batch element (512 tokens per core). Layer weights are replicated per core and
streamed from HBM; attention does a per-layer K/V all-gather within each
4-core batch group. The lm_head is sharded over vocab (4000 cols per core)
after an 8-rank all-gather of the final hidden states.

Activations are kept transposed on-chip: hT[d, t] with d on partitions, so all
projections consume weight tiles as lhsT directly and the token count (512) is
the matmul free dim. Softmax runs without max-subtraction (scores are O(1) for
this model); denominators come from a ones-column appended to V (via its bias
term), so the softmax sum falls out of the ctx matmul for free.
"""

import numpy as np
import ml_dtypes

BF16 = ml_dtypes.bfloat16
MM_BF16 = True   # bf16 matmul operands (fp32 accumulation everywhere)

L, D, H, F, V = 4, 1024, 16, 4096, 32000
B, S = 2, 2048
HD = D // H          # 64
NCORES = 8
TL = 512             # tokens per core
P = 128
DI = D // P          # 8 d-chunks
FC = F // P          # 32 f-chunks
KC = S // P          # 16 key chunks per batch
VSH = V // NCORES    # 4000 vocab cols per core
VPAD = 4096
NV = VPAD // 512     # 8 n-chunks of lm head per core
EPS = 1e-5
SCALE = 1.0 / np.sqrt(HD)
VA = 65              # per-head v width incl. ones column
VCH = [(0, 7), (7, 7), (14, 2)]   # head-aligned v column chunks (h0, nheads)
GROUPS4 = [[0, 1, 2, 3], [4, 5, 6, 7]]
GROUPS8 = [list(range(8))]

_CACHE = {}


def _pack_weights(embed, pe, wq, bq, wk, bk, wv, bv, wo, bo,
                  ln1_g, ln1_b, w1, b1, w2, b2, ln2_g, ln2_b, lnf_g, lnf_b,
                  lm_head, input_ids):
    """Host-side packing of weights into tile layouts.
    Returns (shared inputs, per-core inputs, flags)."""
    f32 = np.float32
    asf = lambda x: np.asarray(x, dtype=f32)
    wq, bq, wk, bk, wv, bv, wo, bo = map(asf, (wq, bq, wk, bk, wv, bv, wo, bo))
    w1, b1, w2, b2 = map(asf, (w1, b1, w2, b2))
    ln1_g, ln1_b, ln2_g, ln2_b = map(asf, (ln1_g, ln1_b, ln2_g, ln2_b))
    lnf_g, lnf_b = asf(lnf_g), asf(lnf_b)

    def lhst_tiles(w):
        # w [K, M] -> [M/P, P(k), K/P, P(m)]: row-block layout matching the
        # SBUF tile [P, K/P, P] so the DMA is a plain contiguous copy
        Kd, M = w.shape
        t = w.reshape(Kd // P, P, M // P, P)          # ki, pk, mi, pm
        return np.ascontiguousarray(t.transpose(2, 1, 0, 3))

    wdt = BF16 if MM_BF16 else f32
    shared = {}
    shared['wqp'] = np.stack([lhst_tiles(wq[l]) for l in range(L)]).astype(wdt)
    shared['wkp'] = np.stack([lhst_tiles(wk[l]) for l in range(L)]).astype(wdt)
    shared['wop'] = np.stack([lhst_tiles(wo[l]) for l in range(L)]).astype(wdt)
    shared['w1p'] = np.stack([lhst_tiles(w1[l]) for l in range(L)]).astype(wdt)
    shared['w2p'] = np.stack([lhst_tiles(w2[l]) for l in range(L)]).astype(wdt)

    # v augmented: per head 64 cols of wv + 1 zero col; bias gets 1.0 there
    wv_aug = np.zeros((L, D, H * VA), dtype=f32)
    bv_aug = np.zeros((L, H * VA), dtype=f32)
    for h in range(H):
        wv_aug[:, :, h * VA:h * VA + HD] = wv[:, :, h * HD:(h + 1) * HD]
        bv_aug[:, h * VA:h * VA + HD] = bv[:, h * HD:(h + 1) * HD]
        bv_aug[:, h * VA + HD] = 1.0
    shared['wvp'] = np.ascontiguousarray(
        wv_aug.reshape(L, DI, P, H * VA)).astype(wdt)
    shared['bvp'] = bv_aug.reshape(L, 1, H * VA)

    # per-partition biases packed [L, P, nch]
    pk = lambda b, n: np.ascontiguousarray(
        b.reshape(L, n, P).transpose(0, 2, 1))
    shared['bqp'] = pk(bq, DI)
    shared['bkp'] = pk(bk, DI)
    shared['b1p'] = pk(b1, FC)
    shared['bop'] = pk(bo, DI)
    shared['b2p'] = b2.reshape(L, 1, DI, P)   # lhsT [1, P] tiles for bias mm

    # LN params: K=2 lhsT tiles [nln, di, 2(b,g), P] + per-partition g
    lnp = np.zeros((2 * L + 1, DI, 2, P), dtype=f32)
    lng = np.zeros((2 * L + 1, P, DI), dtype=f32)
    for l in range(L):
        for j, (g, b) in enumerate(((ln1_g, ln1_b), (ln2_g, ln2_b))):
            lnp[2 * l + j, :, 0, :] = b[l].reshape(DI, P)
            lnp[2 * l + j, :, 1, :] = g[l].reshape(DI, P)
            lng[2 * l + j] = g[l].reshape(DI, P).T
    lnp[2 * L, :, 0, :] = lnf_b.reshape(DI, P)
    lnp[2 * L, :, 1, :] = lnf_g.reshape(DI, P)
    lng[2 * L] = lnf_g.reshape(DI, P).T
    shared['lnp'] = lnp
    shared['lng'] = lng

    flags = {'bo': bool(np.any(bo)), 'b2': bool(np.any(b2))}

    # per-core: tokT, peT, lm_head shard
    per_core = []
    ids = np.asarray(input_ids).astype(np.int64)
    emb = asf(embed)
    pe_np = asf(pe)
    lmh = asf(lm_head)
    for r in range(NCORES):
        b_, q_ = r // 4, r % 4
        tok = emb[ids[b_, q_ * TL:(q_ + 1) * TL]]            # [TL, D]
        tokT = np.ascontiguousarray(tok.T).reshape(DI, P, TL)
        peT = np.ascontiguousarray(
            pe_np[q_ * TL:(q_ + 1) * TL, :].T).reshape(DI, P, TL)
        slp = np.zeros((D, VPAD), dtype=f32)
        slp[:, :VSH] = lmh[:, r * VSH:(r + 1) * VSH]
        lmt = np.ascontiguousarray(
            slp.reshape(DI, P, NV, 512).transpose(2, 1, 0, 3)).astype(wdt)
        per_core.append({'tokT': tokT, 'peT': peT, 'lmhp': lmt})
    return shared, per_core, flags


def _build(flags):
    import concourse.bass as bass  # noqa: F401
    import concourse.tile as tile
    from concourse import bacc, mybir
    from contextlib import ExitStack

    dt = mybir.dt.float32
    dtb = mybir.dt.bfloat16 if MM_BF16 else dt
    AF = mybir.ActivationFunctionType
    OP = mybir.AluOpType

    nc = bacc.Bacc("TRN2", target_bir_lowering=False, debug=False,
                   num_devices=NCORES)

    def din(name, shape, d=None):
        return nc.dram_tensor(name, list(shape), d or dt,
                              kind="ExternalInput").ap()

    tokT_d = din('tokT', (DI, P, TL))
    peT_d = din('peT', (DI, P, TL))
    wqp = din('wqp', (L, DI, P, DI, P), dtb)
    wkp = din('wkp', (L, DI, P, DI, P), dtb)
    wop = din('wop', (L, DI, P, DI, P), dtb)
    w1p = din('w1p', (L, FC, P, DI, P), dtb)
    w2p = din('w2p', (L, DI, P, FC, P), dtb)
    wvp = din('wvp', (L, DI, P, H * VA), dtb)
    bvp = din('bvp', (L, 1, H * VA))
    bqp = din('bqp', (L, P, DI))
    bkp = din('bkp', (L, P, DI))
    b1p = din('b1p', (L, P, FC))
    bop = din('bop', (L, P, DI))
    b2p = din('b2p', (L, 1, DI, P))
    lnp_d = din('lnp', (2 * L + 1, DI, 2, P))
    lng_d = din('lng', (2 * L + 1, P, DI))
    lmhp = din('lmhp', (NV, P, DI, 512), dtb)
    logits_d = nc.dram_tensor('logits', [NCORES * TL, VPAD], dt,
                              kind="ExternalOutput").ap()

    with tile.TileContext(nc) as tc, ExitStack() as top:
        const = top.enter_context(tc.tile_pool(name="const", bufs=1))
        ones_col = const.tile([P, 1], dt)      # lhsT for partition-sum MMs
        nc.vector.memset(ones_col, 1.0)
        ones_row = const.tile([1, P], dt)      # lhsT for K=1 broadcast MMs
        nc.vector.memset(ones_row, 1.0)
        ones512 = const.tile([1, TL], dt)      # rhs for per-partition bias MMs
        nc.vector.memset(ones512, 1.0)
        eps_t = const.tile([1, 1], dt)
        nc.vector.memset(eps_t, EPS)

        hpool = top.enter_context(tc.tile_pool(name="h", bufs=1))
        hT = hpool.tile([P, DI, TL], dt, tag="hT")
        hT_bf = hpool.tile([P, DI, TL], dtb, tag="hTbf")

        def cast_h(dcs=range(DI)):
            for dc in dcs:
                nc.vector.tensor_copy(hT_bf[:, dc, :], hT[:, dc, :])

        # one psum pool; tags: mm(2) + sc(2) + ctx(2) + small(2) = 8 banks
        pp = top.enter_context(tc.tile_pool(name="pp", bufs=2, space="PSUM"))
        dram = top.enter_context(tc.tile_pool(name="dram", bufs=2,
                                              space="DRAM"))

        def layernorm(x, ln_idx, scr, biasp):
            """In-place layernorm over d (partitions x chunks) of x."""
            ps_mean = pp.tile([1, TL], dt, tag="small")
            for dc in range(DI):
                nc.tensor.matmul(ps_mean, ones_col, x[:, dc, :],
                                 start=(dc == 0), stop=(dc == DI - 1))
            ps_sq = pp.tile([1, TL], dt, tag="small")
            for dc in range(DI):
                sq = scr.tile([P, TL], dt, tag="lnt")
                nc.scalar.activation(sq, x[:, dc, :], AF.Square)
                nc.tensor.matmul(ps_sq, ones_col, sq,
                                 start=(dc == 0), stop=(dc == DI - 1))
            m = scr.tile([1, TL], dt, tag="m", bufs=1)       # -mean
            nc.scalar.activation(m, ps_mean, AF.Copy, scale=-1.0 / D)
            var = scr.tile([1, TL], dt, tag="var", bufs=1)
            nc.vector.tensor_tensor(var, m, m, OP.mult)          # m^2
            msq = scr.tile([1, TL], dt, tag="msq", bufs=1)
            nc.scalar.activation(msq, ps_sq, AF.Copy, scale=1.0 / D)
            nc.vector.tensor_tensor(var, msq, var, OP.subtract)  # variance
            nc.scalar.activation(var, var, AF.Sqrt, bias=eps_t)  # stddev
            s = scr.tile([1, TL], dt, tag="s", bufs=1)
            nc.vector.reciprocal(s, var)                         # rstd
            u = scr.tile([1, TL], dt, tag="u", bufs=1)           # -m*s
            nc.vector.tensor_tensor(u, m, s, OP.mult)
            ps_s = pp.tile([P, TL], dt, tag="sc")      # s bcast over parts
            nc.tensor.matmul(ps_s, ones_row, s, start=True, stop=True)
            lgt = biasp.tile([P, DI], dt, tag="lng")
            nc.sync.dma_start(lgt, lng_d[ln_idx])
            for dc in range(DI):
                lnb = biasp.tile([1, P], dt, tag="lnb", bufs=2)
                nc.sync.dma_start(lnb, lnp_d[ln_idx, dc, 0:1, :])
                lngr = biasp.tile([1, P], dt, tag="lngr", bufs=2)
                nc.sync.dma_start(lngr, lnp_d[ln_idx, dc, 1:2, :])
                ps_b = pp.tile([P, TL], dt, tag="small")
                nc.tensor.matmul(ps_b, lnb, ones512, start=True, stop=False)
                nc.tensor.matmul(ps_b, lngr, u, start=False, stop=True)
                t = scr.tile([P, TL], dt, tag="lnt")
                nc.vector.tensor_tensor(t, x[:, dc, :], ps_s, OP.mult)
                nc.vector.scalar_tensor_tensor(
                    x[:, dc, :], t, lgt[:, dc:dc + 1], ps_b, OP.mult, OP.add)
                nc.vector.tensor_copy(hT_bf[:, dc, :], x[:, dc, :])

        with ExitStack() as body:
            wrow = body.enter_context(tc.tile_pool(name="wrow", bufs=3))
            w2row = body.enter_context(tc.tile_pool(name="w2row", bufs=2))
            wvsl = body.enter_context(tc.tile_pool(name="wvsl", bufs=8))
            qpool = body.enter_context(tc.tile_pool(name="q", bufs=1))
            cpool = body.enter_context(tc.tile_pool(name="ctx", bufs=1))
            kpool = body.enter_context(tc.tile_pool(name="khp", bufs=2))
            epool = body.enter_context(tc.tile_pool(name="expp", bufs=2))
            vpool = body.enter_context(tc.tile_pool(name="vt", bufs=4))
            gpool = body.enter_context(tc.tile_pool(name="g", bufs=1))
            scr = body.enter_context(tc.tile_pool(name="scr", bufs=2))
            biasp = body.enter_context(tc.tile_pool(name="bias", bufs=2))

            # h0 = 2*tok + pe (transposed layout)
            for dc in range(DI):
                tk = scr.tile([P, TL], dt, tag="kd")
                nc.sync.dma_start(tk, tokT_d[dc])
                pec = scr.tile([P, TL], dt, tag="lnt")
                nc.sync.dma_start(pec, peT_d[dc])
                nc.vector.tensor_scalar(hT[:, dc, :], tk, 2.0, None, OP.mult)
                nc.vector.tensor_tensor(hT[:, dc, :], hT[:, dc, :], pec,
                                        OP.add)
            cast_h()

            for l in range(L):
                # ---- K projection -> DRAM -> AllGather ----
                kT_loc = dram.tile([DI, P, TL], dtb, tag="kloc")
                bk_sb = biasp.tile([P, DI], dt, tag="bk")
                nc.sync.dma_start(bk_sb, bkp[l])
                for dc in range(DI):
                    wr = wrow.tile([P, DI, P], dtb, tag="wr")
                    nc.sync.dma_start(wr, wkp[l, dc])
                    ps = pp.tile([P, TL], dt, tag="mm")
                    for di in range(DI):
                        nc.tensor.matmul(ps, wr[:, di, :], hT_bf[:, di, :],
                                         start=(di == 0), stop=(di == DI - 1))
                    kd = scr.tile([P, TL], dtb, tag="kd")
                    nc.scalar.activation(kd, ps, AF.Identity,
                                         bias=bk_sb[:, dc:dc + 1])
                    nc.sync.dma_start(kT_loc[dc], kd)
                kT_full = dram.tile([4, DI, P, TL], dtb, tag="kfull")
                nc.gpsimd.collective_compute(
                    "AllGather", OP.bypass,
                    ins=[kT_loc.opt()], outs=[kT_full.opt()],
                    replica_groups=GROUPS4)

                # ---- V projection (token-major, augmented cols) ----
                v_loc = dram.tile([H, TL, VA], dtb, tag="vloc")
                for (h0, nh) in VCH:
                    c0, cw = h0 * VA, nh * VA
                    bv_sb = biasp.tile([1, 7 * VA], dt, tag="bv", bufs=2)
                    nc.sync.dma_start(bv_sb[:, :cw], bvp[l][:, c0:c0 + cw])
                    wv_tiles = []
                    for di in range(DI):
                        wt = wvsl.tile([P, 7 * VA], dtb, tag="wv")
                        nc.sync.dma_start(wt[:, :cw], wvp[l, di, :, c0:c0 + cw])
                        wv_tiles.append(wt)
                    for t4 in range(4):
                        ps = pp.tile([P, 7 * VA], dt, tag="mm")
                        for di in range(DI):
                            nc.tensor.matmul(
                                ps[:, :cw],
                                hT_bf[:, di, t4 * P:(t4 + 1) * P],
                                wv_tiles[di][:, :cw],
                                start=(di == 0), stop=False)
                        nc.tensor.matmul(ps[:, :cw], ones_row,
                                         bv_sb[:, :cw],
                                         start=False, stop=True)
                        vd = scr.tile([P, TL], dtb, tag="kd")
                        nc.scalar.activation(vd[:, :cw], ps[:, :cw], AF.Copy)
                        for j in range(nh):
                            nc.sync.dma_start(
                                v_loc[h0 + j, t4 * P:(t4 + 1) * P, :],
                                vd[:, j * VA:(j + 1) * VA])
                v_full = dram.tile([4, H, TL, VA], dtb, tag="vfull")
                nc.gpsimd.collective_compute(
                    "AllGather", OP.bypass,
                    ins=[v_loc.opt()], outs=[v_full.opt()],
                    replica_groups=GROUPS4)

                # ---- Q projection (stays in SBUF) ----
                qT = qpool.tile([P, DI, TL], dtb, tag="qT")
                bq_sb = biasp.tile([P, DI], dt, tag="bq")
                nc.sync.dma_start(bq_sb, bqp[l])
                for dc in range(DI):
                    wr = wrow.tile([P, DI, P], dtb, tag="wr")
                    nc.sync.dma_start(wr, wqp[l, dc])
                    ps = pp.tile([P, TL], dt, tag="mm")
                    for di in range(DI):
                        nc.tensor.matmul(ps, wr[:, di, :], hT_bf[:, di, :],
                                         start=(di == 0), stop=(di == DI - 1))
                    nc.scalar.activation(qT[:, dc, :], ps, AF.Identity,
                                         bias=bq_sb[:, dc:dc + 1])

                # ---- attention ----
                ctxT = cpool.tile([P, DI, TL], dtb, tag="ctxT")
                for hp in range(H // 2):
                    khp = kpool.tile([P, 4, TL], dtb, tag="khp")
                    for r in range(4):
                        nc.sync.dma_start(khp[:, r, :], kT_full[r, hp])
                    for hh in range(2):
                        h = hp * 2 + hh
                        pb = hh * HD
                        ps_ctx = pp.tile([VA, TL], dt, tag="ctx")
                        for kc in range(KC):
                            ps_sc = pp.tile([P, TL], dt, tag="sc")
                            nc.tensor.matmul(
                                ps_sc,
                                khp[pb:pb + HD, kc // 4,
                                    (kc % 4) * P:(kc % 4 + 1) * P],
                                qT[pb:pb + HD, hp, :],
                                start=True, stop=True)
                            ex = epool.tile([P, TL], dtb, tag="ex")
                            nc.scalar.activation(ex, ps_sc, AF.Exp,
                                                 scale=SCALE)
                            vt = vpool.tile([P, VA], dtb, tag="vt")
                            nc.sync.dma_start(
                                vt,
                                v_full[kc // 4, h,
                                       (kc % 4) * P:(kc % 4 + 1) * P, :])
                            nc.tensor.matmul(ps_ctx, vt, ex,
                                             start=(kc == 0),
                                             stop=(kc == KC - 1))
                        rec = scr.tile([1, TL], dt, tag="rec", bufs=2)
                        nc.vector.reciprocal(rec, ps_ctx[HD:HD + 1, :])
                        ps_bc = pp.tile([HD, TL], dt, tag="small")
                        nc.tensor.matmul(ps_bc, ones_row[:, :HD], rec,
                                         start=True, stop=True)
                        nc.scalar.activation(ctxT[pb:pb + HD, hp, :],
                                             ps_ctx[0:HD, :], AF.Copy)
                        nc.vector.tensor_tensor(
                            ctxT[pb:pb + HD, hp, :],
                            ctxT[pb:pb + HD, hp, :], ps_bc, OP.mult)

                # ---- O projection + residual ----
                bo_sb = None
                if flags['bo']:
                    bo_sb = biasp.tile([P, DI], dt, tag="bo")
                    nc.sync.dma_start(bo_sb, bop[l])
                for dc in range(DI):
                    wr = wrow.tile([P, DI, P], dtb, tag="wr")
                    nc.sync.dma_start(wr, wop[l, dc])
                    ps = pp.tile([P, TL], dt, tag="mm")
                    for di in range(DI):
                        nc.tensor.matmul(ps, wr[:, di, :], ctxT[:, di, :],
                                         start=(di == 0), stop=(di == DI - 1))
                    if bo_sb is not None:
                        nc.vector.scalar_tensor_tensor(
                            hT[:, dc, :], ps, bo_sb[:, dc:dc + 1],
                            hT[:, dc, :], OP.add, OP.add)
                    else:
                        nc.vector.tensor_tensor(hT[:, dc, :], ps,
                                                hT[:, dc, :], OP.add)
                layernorm(hT, 2 * l, scr, biasp)

                # ---- FFN ----
                gT = gpool.tile([P, FC, TL], dtb, tag="gT")
                b1_sb = biasp.tile([P, FC], dt, tag="b1")
                nc.sync.dma_start(b1_sb, b1p[l])
                for fc in range(FC):
                    wr = wrow.tile([P, DI, P], dtb, tag="wr")
                    nc.sync.dma_start(wr, w1p[l, fc])
                    ps = pp.tile([P, TL], dt, tag="mm")
                    for di in range(DI):
                        nc.tensor.matmul(ps, wr[:, di, :], hT_bf[:, di, :],
                                         start=(di == 0), stop=(di == DI - 1))
                    nc.scalar.activation(gT[:, fc, :], ps, AF.Gelu,
                                         bias=b1_sb[:, fc:fc + 1])
                b2_sb = None
                if flags['b2']:
                    b2_sb = biasp.tile([1, DI, P], dt, tag="b2")
                    nc.sync.dma_start(b2_sb, b2p[l])
                for dc in range(DI):
                    ps = pp.tile([P, TL], dt, tag="sc")
                    for half in range(2):
                        w2r = w2row.tile([P, FC // 2, P], dtb, tag="w2r")
                        nc.sync.dma_start(
                            w2r,
                            w2p[l, dc][:, half * FC // 2:(half + 1) * FC // 2,
                                       :])
                        for j in range(FC // 2):
                            fc = half * FC // 2 + j
                            nc.tensor.matmul(
                                ps, w2r[:, j, :], gT[:, fc, :],
                                start=(fc == 0),
                                stop=(fc == FC - 1 and b2_sb is None))
                    if b2_sb is not None:
                        nc.tensor.matmul(ps, b2_sb[:, dc, :], ones512,
                                         start=False, stop=True)
                    nc.vector.tensor_tensor(hT[:, dc, :], ps, hT[:, dc, :],
                                            OP.add)
                layernorm(hT, 2 * l + 1, scr, biasp)

            # final LN + store h for all-gather
            layernorm(hT, 2 * L, scr, biasp)
            hf_loc = dram.tile([DI, P, TL], dtb, tag="hfloc")
            for dc in range(DI):
                nc.sync.dma_start(hf_loc[dc], hT_bf[:, dc, :])
            hf_full = dram.tile([NCORES, DI, P, TL], dtb, tag="hffull")
            nc.gpsimd.collective_compute(
                "AllGather", OP.bypass,
                ins=[hf_loc.opt()], outs=[hf_full.opt()],
                replica_groups=GROUPS8)

        # ---- lm head (vocab shard) ----
        with ExitStack() as lmctx:
            hfp = lmctx.enter_context(tc.tile_pool(name="hf", bufs=1))
            lmp = lmctx.enter_context(tc.tile_pool(name="lmw", bufs=2))
            outp = lmctx.enter_context(tc.tile_pool(name="lout", bufs=4))
            hf_sb = hfp.tile([P, DI, NCORES, TL], dtb, tag="hf")
            for r in range(NCORES):
                nc.sync.dma_start(hf_sb[:, :, r, :],
                                  hf_full[r].rearrange("d p t -> p d t"))
            for vc in range(NV):
                lmr = lmp.tile([P, DI, 512], dtb, tag="lmr")
                nc.sync.dma_start(lmr, lmhp[vc])
                for tcn in range(NCORES * TL // P):
                    r, lt = tcn // 4, tcn % 4
                    ps = pp.tile([P, 512], dt, tag="mm")
                    for di in range(DI):
                        nc.tensor.matmul(
                            ps,
                            hf_sb[:, di, r, lt * P:(lt + 1) * P],
                            lmr[:, di, :],
                            start=(di == 0), stop=(di == DI - 1))
                    ot = outp.tile([P, 512], dt, tag="ot")
                    nc.scalar.activation(ot, ps, AF.Copy)
                    nc.sync.dma_start(
                        logits_d[tcn * P:(tcn + 1) * P,
                                 vc * 512:(vc + 1) * 512], ot)

    nc.compile()
    return nc


def kernel(**inputs):
    from concourse.bass_utils import run_bass_kernel_spmd

    shared, per_core, flags = _pack_weights(**inputs)
    key = tuple(sorted(flags.items()))
    if key not in _CACHE:
        _CACHE[key] = _build(flags)
    nc = _CACHE[key]

    in_maps = []
    for r in range(NCORES):
        m = dict(shared)
        m.update(per_core[r])
        in_maps.append(m)
    res = run_bass_kernel_spmd(nc, in_maps, list(range(NCORES)))
    logits = np.zeros((B, S, V), dtype=np.float32)
    flat = logits.reshape(B * S, V)
    for r in range(NCORES):
        flat[:, r * VSH:(r + 1) * VSH] = res.results[r]['logits'][:, :VSH]
    return logits


# revision 16
# speedup vs baseline: 2.4661x; 1.0372x over previous
"""Trainium2 Bass kernel for a 4-layer dense transformer (B=2, S=2048, D=1024,
H=16, F=4096, V=32000) running on 8 NeuronCores.

Sharding: 2-way data parallel over batch x 4-way sequence sharding within each
batch element (512 tokens per core). Layer weights are replicated per core and
streamed from HBM; attention does a per-layer K/V all-gather within each
4-core batch group. The lm_head is sharded over vocab (4000 cols per core)
after an 8-rank all-gather of the final hidden states.

Activations are kept transposed on-chip: hT[d, t] with d on partitions, so all
projections consume weight tiles as lhsT directly and the token count (512) is
the matmul free dim. Softmax runs without max-subtraction (scores are O(1) for
this model); denominators come from a ones-column appended to V (via its bias
term), so the softmax sum falls out of the ctx matmul for free.
"""

import numpy as np
import ml_dtypes

BF16 = ml_dtypes.bfloat16
MM_BF16 = True   # bf16 matmul operands (fp32 accumulation everywhere)

L, D, H, F, V = 4, 1024, 16, 4096, 32000
B, S = 2, 2048
HD = D // H          # 64
NCORES = 8
TL = 512             # tokens per core
P = 128
DI = D // P          # 8 d-chunks
FC = F // P          # 32 f-chunks
KC = S // P          # 16 key chunks per batch
VSH = V // NCORES    # 4000 vocab cols per core
VPAD = 4096
NV = VPAD // 512     # 8 n-chunks of lm head per core
EPS = 1e-5
SCALE = 1.0 / np.sqrt(HD)
VA = 65              # per-head v width incl. ones column
VCH = [(0, 7), (7, 7), (14, 2)]   # head-aligned v column chunks (h0, nheads)
GROUPS4 = [[0, 1, 2, 3], [4, 5, 6, 7]]
GROUPS8 = [list(range(8))]

_CACHE = {}


def _pack_weights(embed, pe, wq, bq, wk, bk, wv, bv, wo, bo,
                  ln1_g, ln1_b, w1, b1, w2, b2, ln2_g, ln2_b, lnf_g, lnf_b,
                  lm_head, input_ids):
    """Host-side packing of weights into tile layouts.
    Returns (shared inputs, per-core inputs, flags)."""
    f32 = np.float32
    asf = lambda x: np.asarray(x, dtype=f32)
    wq, bq, wk, bk, wv, bv, wo, bo = map(asf, (wq, bq, wk, bk, wv, bv, wo, bo))
    w1, b1, w2, b2 = map(asf, (w1, b1, w2, b2))
    ln1_g, ln1_b, ln2_g, ln2_b = map(asf, (ln1_g, ln1_b, ln2_g, ln2_b))
    lnf_g, lnf_b = asf(lnf_g), asf(lnf_b)

    def lhst_tiles(w):
        # w [K, M] -> [M/P, P(k), K/P, P(m)]: row-block layout matching the
        # SBUF tile [P, K/P, P] so the DMA is a plain contiguous copy
        Kd, M = w.shape
        t = w.reshape(Kd // P, P, M // P, P)          # ki, pk, mi, pm
        return np.ascontiguousarray(t.transpose(2, 1, 0, 3))

    wdt = BF16 if MM_BF16 else f32
    shared = {}
    shared['wqp'] = np.stack([lhst_tiles(wq[l]) for l in range(L)]).astype(wdt)
    shared['wkp'] = np.stack([lhst_tiles(wk[l]) for l in range(L)]).astype(wdt)
    shared['wop'] = np.stack([lhst_tiles(wo[l]) for l in range(L)]).astype(wdt)
    shared['w1p'] = np.stack([lhst_tiles(w1[l]) for l in range(L)]).astype(wdt)
    shared['w2p'] = np.stack([lhst_tiles(w2[l]) for l in range(L)]).astype(wdt)

    # v augmented: per head 64 cols of wv + 1 zero col; bias gets 1.0 there
    wv_aug = np.zeros((L, D, H * VA), dtype=f32)
    bv_aug = np.zeros((L, H * VA), dtype=f32)
    for h in range(H):
        wv_aug[:, :, h * VA:h * VA + HD] = wv[:, :, h * HD:(h + 1) * HD]
        bv_aug[:, h * VA:h * VA + HD] = bv[:, h * HD:(h + 1) * HD]
        bv_aug[:, h * VA + HD] = 1.0
    shared['wvp'] = np.ascontiguousarray(
        wv_aug.reshape(L, DI, P, H * VA)).astype(wdt)
    shared['bvp'] = bv_aug.reshape(L, 1, H * VA)

    # per-partition biases packed [L, P, nch]
    pk = lambda b, n: np.ascontiguousarray(
        b.reshape(L, n, P).transpose(0, 2, 1))
    shared['bqp'] = pk(bq, DI)
    shared['bkp'] = pk(bk, DI)
    shared['b1p'] = pk(b1, FC)
    shared['bop'] = pk(bo, DI)
    shared['b2p'] = b2.reshape(L, 1, DI, P)   # lhsT [1, P] tiles for bias mm

    # LN params: K=2 lhsT tiles [nln, di, 2(b,g), P] + per-partition g
    lnp = np.zeros((2 * L + 1, DI, 2, P), dtype=f32)
    lng = np.zeros((2 * L + 1, P, DI), dtype=f32)
    for l in range(L):
        for j, (g, b) in enumerate(((ln1_g, ln1_b), (ln2_g, ln2_b))):
            lnp[2 * l + j, :, 0, :] = b[l].reshape(DI, P)
            lnp[2 * l + j, :, 1, :] = g[l].reshape(DI, P)
            lng[2 * l + j] = g[l].reshape(DI, P).T
    lnp[2 * L, :, 0, :] = lnf_b.reshape(DI, P)
    lnp[2 * L, :, 1, :] = lnf_g.reshape(DI, P)
    lng[2 * L] = lnf_g.reshape(DI, P).T
    shared['lnp'] = lnp
    shared['lng'] = lng

    flags = {'bo': bool(np.any(bo)), 'b2': bool(np.any(b2))}

    # per-core: tokT, peT, lm_head shard
    per_core = []
    ids = np.asarray(input_ids).astype(np.int64)
    emb = asf(embed)
    pe_np = asf(pe)
    lmh = asf(lm_head)
    for r in range(NCORES):
        b_, q_ = r // 4, r % 4
        tok = emb[ids[b_, q_ * TL:(q_ + 1) * TL]]            # [TL, D]
        tokT = np.ascontiguousarray(tok.T).reshape(DI, P, TL)
        peT = np.ascontiguousarray(
            pe_np[q_ * TL:(q_ + 1) * TL, :].T).reshape(DI, P, TL)
        slp = np.zeros((D, VPAD), dtype=f32)
        slp[:, :VSH] = lmh[:, r * VSH:(r + 1) * VSH]
        lmt = np.ascontiguousarray(
            slp.reshape(DI, P, NV, 512).transpose(2, 1, 0, 3)).astype(wdt)
        per_core.append({'tokT': tokT, 'peT': peT, 'lmhp': lmt})
    return shared, per_core, flags


def _build(flags):
    import concourse.bass as bass  # noqa: F401
    import concourse.tile as tile
    from concourse import bacc, mybir
    from contextlib import ExitStack

    dt = mybir.dt.float32
    dtb = mybir.dt.bfloat16 if MM_BF16 else dt
    AF = mybir.ActivationFunctionType
    OP = mybir.AluOpType

    nc = bacc.Bacc("TRN2", target_bir_lowering=False, debug=False,
                   num_devices=NCORES)

    def din(name, shape, d=None):
        return nc.dram_tensor(name, list(shape), d or dt,
                              kind="ExternalInput").ap()

    tokT_d = din('tokT', (DI, P, TL))
    peT_d = din('peT', (DI, P, TL))
    wqp = din('wqp', (L, DI, P, DI, P), dtb)
    wkp = din('wkp', (L, DI, P, DI, P), dtb)
    wop = din('wop', (L, DI, P, DI, P), dtb)
    w1p = din('w1p', (L, FC, P, DI, P), dtb)
    w2p = din('w2p', (L, DI, P, FC, P), dtb)
    wvp = din('wvp', (L, DI, P, H * VA), dtb)
    bvp = din('bvp', (L, 1, H * VA))
    bqp = din('bqp', (L, P, DI))
    bkp = din('bkp', (L, P, DI))
    b1p = din('b1p', (L, P, FC))
    bop = din('bop', (L, P, DI))
    b2p = din('b2p', (L, 1, DI, P))
    lnp_d = din('lnp', (2 * L + 1, DI, 2, P))
    lng_d = din('lng', (2 * L + 1, P, DI))
    lmhp = din('lmhp', (NV, P, DI, 512), dtb)
    logits_d = nc.dram_tensor('logits', [NCORES * TL, VPAD], dt,
                              kind="ExternalOutput").ap()

    with tile.TileContext(nc) as tc, ExitStack() as top:
        const = top.enter_context(tc.tile_pool(name="const", bufs=1))
        ones_col = const.tile([P, 1], dt)      # lhsT for partition-sum MMs
        nc.vector.memset(ones_col, 1.0)
        ones_row = const.tile([1, P], dt)      # lhsT for K=1 broadcast MMs
        nc.vector.memset(ones_row, 1.0)
        ones128 = const.tile([P, P], dt)       # lhsT for replicated part-sums
        nc.vector.memset(ones128, 1.0)
        ones512 = const.tile([1, TL], dt)      # rhs for per-partition bias MMs
        nc.vector.memset(ones512, 1.0)
        eps_col = const.tile([P, 1], dt)
        nc.vector.memset(eps_col, EPS)

        hpool = top.enter_context(tc.tile_pool(name="h", bufs=1))
        hT = hpool.tile([P, DI, TL], dt, tag="hT")
        hT_bf = hpool.tile([P, DI, TL], dtb, tag="hTbf")

        def cast_h(dcs=range(DI)):
            for dc in dcs:
                nc.vector.tensor_copy(hT_bf[:, dc, :], hT[:, dc, :])

        # one psum pool; tags: mm(2) + sc(2) + ctx(2) + small(2) = 8 banks
        pp = top.enter_context(tc.tile_pool(name="pp", bufs=2, space="PSUM"))
        dram = top.enter_context(tc.tile_pool(name="dram", bufs=2,
                                              space="DRAM"))

        def layernorm(x, ln_idx, scr, biasp):
            """In-place layernorm over d (partitions x chunks) of x.

            Stats are computed replicated across all 128 partitions (lhsT of
            ones [128,128]) so every elementwise stat op runs full-width."""
            ps_mean = pp.tile([P, TL], dt, tag="small")
            for dc in range(DI):
                nc.tensor.matmul(ps_mean, ones128, x[:, dc, :],
                                 start=(dc == 0), stop=(dc == DI - 1))
            ps_sq = pp.tile([P, TL], dt, tag="small")
            for dc in range(DI):
                sq = scr.tile([P, TL], dt, tag="lnt")
                if dc % 2 == 0:
                    nc.scalar.activation(sq, x[:, dc, :], AF.Square)
                else:
                    nc.vector.tensor_tensor(sq, x[:, dc, :], x[:, dc, :],
                                            OP.mult)
                nc.tensor.matmul(ps_sq, ones128, sq,
                                 start=(dc == 0), stop=(dc == DI - 1))
            m_b = scr.tile([P, TL], dt, tag="mb", bufs=2)    # -mean, wide
            nc.scalar.activation(m_b, ps_mean, AF.Copy, scale=-1.0 / D)
            var = scr.tile([P, TL], dt, tag="varb", bufs=2)
            nc.vector.tensor_tensor(var, m_b, m_b, OP.mult)  # mean^2
            nc.vector.scalar_tensor_tensor(var, ps_sq, 1.0 / D, var,
                                           OP.mult, OP.subtract)
            nc.scalar.activation(var, var, AF.Sqrt,
                                 bias=eps_col)               # stddev, wide
            s_b = scr.tile([P, TL], dt, tag="sb", bufs=2)
            nc.vector.reciprocal(s_b, var)                   # rstd, wide
            u_b = scr.tile([P, TL], dt, tag="ub", bufs=2)    # -mean*rstd
            nc.vector.tensor_tensor(u_b, m_b, s_b, OP.mult)
            lgt = biasp.tile([P, DI], dt, tag="lng")
            nc.sync.dma_start(lgt, lng_d[ln_idx])
            for dc in range(DI):
                lnb = biasp.tile([1, P], dt, tag="lnb", bufs=2)
                nc.sync.dma_start(lnb, lnp_d[ln_idx, dc, 0:1, :])
                lngr = biasp.tile([1, P], dt, tag="lngr", bufs=2)
                nc.sync.dma_start(lngr, lnp_d[ln_idx, dc, 1:2, :])
                ps_b = pp.tile([P, TL], dt, tag="small")
                nc.tensor.matmul(ps_b, lnb, ones512, start=True, stop=False)
                nc.tensor.matmul(ps_b, lngr, u_b[0:1, :], start=False,
                                 stop=True)
                t = scr.tile([P, TL], dt, tag="lnt")
                nc.vector.tensor_tensor(t, x[:, dc, :], s_b, OP.mult)
                nc.vector.scalar_tensor_tensor(
                    x[:, dc, :], t, lgt[:, dc:dc + 1], ps_b, OP.mult, OP.add)
                nc.vector.tensor_copy(hT_bf[:, dc, :], x[:, dc, :])

        with ExitStack() as body:
            wrow = body.enter_context(tc.tile_pool(name="wrow", bufs=6))
            w2row = body.enter_context(tc.tile_pool(name="w2row", bufs=3))
            wvsl = body.enter_context(tc.tile_pool(name="wvsl", bufs=8))
            qpool = body.enter_context(tc.tile_pool(name="q", bufs=1))
            cpool = body.enter_context(tc.tile_pool(name="ctx", bufs=1))
            kpool = body.enter_context(tc.tile_pool(name="khp", bufs=3))
            epool = body.enter_context(tc.tile_pool(name="expp", bufs=6))
            vpool = body.enter_context(tc.tile_pool(name="vt", bufs=12))
            gpool = body.enter_context(tc.tile_pool(name="g", bufs=1))
            scr = body.enter_context(tc.tile_pool(name="scr", bufs=3))
            biasp = body.enter_context(tc.tile_pool(name="bias", bufs=2))

            # h0 = 2*tok + pe (transposed layout)
            for dc in range(DI):
                tk = scr.tile([P, TL], dt, tag="kd")
                nc.sync.dma_start(tk, tokT_d[dc])
                pec = scr.tile([P, TL], dt, tag="lnt")
                nc.sync.dma_start(pec, peT_d[dc])
                nc.vector.tensor_scalar(hT[:, dc, :], tk, 2.0, None, OP.mult)
                nc.vector.tensor_tensor(hT[:, dc, :], hT[:, dc, :], pec,
                                        OP.add)
            cast_h()

            for l in range(L):
                # ---- K projection -> DRAM -> AllGather ----
                kT_loc = dram.tile([DI, P, TL], dtb, tag="kloc")
                bk_sb = biasp.tile([P, DI], dt, tag="bk")
                nc.sync.dma_start(bk_sb, bkp[l])
                for dc in range(DI):
                    wr = wrow.tile([P, DI, P], dtb, tag="wr")
                    nc.sync.dma_start(wr, wkp[l, dc])
                    ps = pp.tile([P, TL], dt, tag="mm", bufs=4)
                    for di in range(DI):
                        nc.tensor.matmul(ps, wr[:, di, :], hT_bf[:, di, :],
                                         start=(di == 0), stop=(di == DI - 1))
                    kd = scr.tile([P, TL], dtb, tag="kd")
                    nc.scalar.activation(kd, ps, AF.Identity,
                                         bias=bk_sb[:, dc:dc + 1])
                    nc.sync.dma_start(kT_loc[dc], kd)
                kT_full = dram.tile([4, DI, P, TL], dtb, tag="kfull")
                nc.gpsimd.collective_compute(
                    "AllGather", OP.bypass,
                    ins=[kT_loc.opt()], outs=[kT_full.opt()],
                    replica_groups=GROUPS4)

                # ---- V projection (token-major, augmented cols) ----
                v_loc = dram.tile([H, TL, VA], dtb, tag="vloc")
                for (h0, nh) in VCH:
                    c0, cw = h0 * VA, nh * VA
                    bv_sb = biasp.tile([1, 7 * VA], dt, tag="bv", bufs=2)
                    nc.sync.dma_start(bv_sb[:, :cw], bvp[l][:, c0:c0 + cw])
                    wv_tiles = []
                    for di in range(DI):
                        wt = wvsl.tile([P, 7 * VA], dtb, tag="wv")
                        nc.sync.dma_start(wt[:, :cw], wvp[l, di, :, c0:c0 + cw])
                        wv_tiles.append(wt)
                    for t4 in range(4):
                        ps = pp.tile([P, 7 * VA], dt, tag="mm", bufs=4)
                        for di in range(DI):
                            nc.tensor.matmul(
                                ps[:, :cw],
                                hT_bf[:, di, t4 * P:(t4 + 1) * P],
                                wv_tiles[di][:, :cw],
                                start=(di == 0), stop=False)
                        nc.tensor.matmul(ps[:, :cw], ones_row,
                                         bv_sb[:, :cw],
                                         start=False, stop=True)
                        vd = scr.tile([P, TL], dtb, tag="kd")
                        nc.scalar.activation(vd[:, :cw], ps[:, :cw], AF.Copy)
                        for j in range(nh):
                            nc.sync.dma_start(
                                v_loc[h0 + j, t4 * P:(t4 + 1) * P, :],
                                vd[:, j * VA:(j + 1) * VA])
                v_full = dram.tile([4, H, TL, VA], dtb, tag="vfull")
                nc.gpsimd.collective_compute(
                    "AllGather", OP.bypass,
                    ins=[v_loc.opt()], outs=[v_full.opt()],
                    replica_groups=GROUPS4)

                # ---- Q projection (stays in SBUF) ----
                qT = qpool.tile([P, DI, TL], dtb, tag="qT")
                bq_sb = biasp.tile([P, DI], dt, tag="bq")
                nc.sync.dma_start(bq_sb, bqp[l])
                for dc in range(DI):
                    wr = wrow.tile([P, DI, P], dtb, tag="wr")
                    nc.sync.dma_start(wr, wqp[l, dc])
                    ps = pp.tile([P, TL], dt, tag="mm", bufs=4)
                    for di in range(DI):
                        nc.tensor.matmul(ps, wr[:, di, :], hT_bf[:, di, :],
                                         start=(di == 0), stop=(di == DI - 1))
                    nc.scalar.activation(qT[:, dc, :], ps, AF.Identity,
                                         bias=bq_sb[:, dc:dc + 1])

                # ---- attention ----
                ctxT = cpool.tile([P, DI, TL], dtb, tag="ctxT")
                for hp in range(H // 2):
                    khp = kpool.tile([P, 4, TL], dtb, tag="khp")
                    for r in range(4):
                        nc.sync.dma_start(khp[:, r, :], kT_full[r, hp])
                    for hh in range(2):
                        h = hp * 2 + hh
                        pb = hh * HD
                        ps_ctx = pp.tile([VA, TL], dt, tag="ctx")
                        for kc in range(KC):
                            ps_sc = pp.tile([P, TL], dt, tag="mm", bufs=4)
                            nc.tensor.matmul(
                                ps_sc,
                                khp[pb:pb + HD, kc // 4,
                                    (kc % 4) * P:(kc % 4 + 1) * P],
                                qT[pb:pb + HD, hp, :],
                                start=True, stop=True)
                            ex = epool.tile([P, TL], dtb, tag="ex")
                            nc.scalar.activation(ex, ps_sc, AF.Exp,
                                                 scale=SCALE)
                            vt = vpool.tile([P, VA], dtb, tag="vt")
                            nc.sync.dma_start(
                                vt,
                                v_full[kc // 4, h,
                                       (kc % 4) * P:(kc % 4 + 1) * P, :])
                            nc.tensor.matmul(ps_ctx, vt, ex,
                                             start=(kc == 0),
                                             stop=(kc == KC - 1))
                        den = scr.tile([1, TL], dt, tag="rec", bufs=2)
                        nc.scalar.activation(den, ps_ctx[HD:HD + 1, :],
                                             AF.Copy)
                        ps_bc = pp.tile([HD, TL], dt, tag="small")
                        nc.tensor.matmul(ps_bc, ones_row[:, :HD], den,
                                         start=True, stop=True)
                        rec = scr.tile([HD, TL], dt, tag="recb", bufs=2)
                        nc.vector.reciprocal(rec, ps_bc)
                        nc.vector.tensor_tensor(
                            ctxT[pb:pb + HD, hp, :],
                            ps_ctx[0:HD, :], rec, OP.mult)

                # ---- O projection + residual ----
                bo_sb = None
                if flags['bo']:
                    bo_sb = biasp.tile([P, DI], dt, tag="bo")
                    nc.sync.dma_start(bo_sb, bop[l])
                for dc in range(DI):
                    wr = wrow.tile([P, DI, P], dtb, tag="wr")
                    nc.sync.dma_start(wr, wop[l, dc])
                    ps = pp.tile([P, TL], dt, tag="mm", bufs=4)
                    for di in range(DI):
                        nc.tensor.matmul(ps, wr[:, di, :], ctxT[:, di, :],
                                         start=(di == 0), stop=(di == DI - 1))
                    if bo_sb is not None:
                        nc.vector.scalar_tensor_tensor(
                            hT[:, dc, :], ps, bo_sb[:, dc:dc + 1],
                            hT[:, dc, :], OP.add, OP.add)
                    else:
                        nc.vector.tensor_tensor(hT[:, dc, :], ps,
                                                hT[:, dc, :], OP.add)
                layernorm(hT, 2 * l, scr, biasp)

                # ---- FFN ----
                gT = gpool.tile([P, FC, TL], dtb, tag="gT")
                b1_sb = biasp.tile([P, FC], dt, tag="b1")
                nc.sync.dma_start(b1_sb, b1p[l])
                for fc in range(FC):
                    wr = wrow.tile([P, DI, P], dtb, tag="wr")
                    nc.sync.dma_start(wr, w1p[l, fc])
                    ps = pp.tile([P, TL], dt, tag="mm", bufs=4)
                    for di in range(DI):
                        nc.tensor.matmul(ps, wr[:, di, :], hT_bf[:, di, :],
                                         start=(di == 0), stop=(di == DI - 1))
                    nc.scalar.activation(gT[:, fc, :], ps, AF.Gelu,
                                         bias=b1_sb[:, fc:fc + 1])
                b2_sb = None
                if flags['b2']:
                    b2_sb = biasp.tile([1, DI, P], dt, tag="b2")
                    nc.sync.dma_start(b2_sb, b2p[l])
                for dc in range(DI):
                    ps = pp.tile([P, TL], dt, tag="mm", bufs=4)
                    for half in range(2):
                        w2r = w2row.tile([P, FC // 2, P], dtb, tag="w2r")
                        nc.sync.dma_start(
                            w2r,
                            w2p[l, dc][:, half * FC // 2:(half + 1) * FC // 2,
                                       :])
                        for j in range(FC // 2):
                            fc = half * FC // 2 + j
                            nc.tensor.matmul(
                                ps, w2r[:, j, :], gT[:, fc, :],
                                start=(fc == 0),
                                stop=(fc == FC - 1 and b2_sb is None))
                    if b2_sb is not None:
                        nc.tensor.matmul(ps, b2_sb[:, dc, :], ones512,
                                         start=False, stop=True)
                    nc.vector.tensor_tensor(hT[:, dc, :], ps, hT[:, dc, :],
                                            OP.add)
                layernorm(hT, 2 * l + 1, scr, biasp)

            # final LN + store h for all-gather
            layernorm(hT, 2 * L, scr, biasp)
            hf_loc = dram.tile([DI, P, TL], dtb, tag="hfloc")
            for dc in range(DI):
                nc.sync.dma_start(hf_loc[dc], hT_bf[:, dc, :])
            hf_full = dram.tile([NCORES, DI, P, TL], dtb, tag="hffull")
            nc.gpsimd.collective_compute(
                "AllGather", OP.bypass,
                ins=[hf_loc.opt()], outs=[hf_full.opt()],
                replica_groups=GROUPS8)

        # ---- lm head (vocab shard) ----
        with ExitStack() as lmctx:
            hfp = lmctx.enter_context(tc.tile_pool(name="hf", bufs=1))
            lmp = lmctx.enter_context(tc.tile_pool(name="lmw", bufs=2))
            outp = lmctx.enter_context(tc.tile_pool(name="lout", bufs=4))
            hf_sb = hfp.tile([P, DI, NCORES, TL], dtb, tag="hf")
            for r in range(NCORES):
                nc.sync.dma_start(hf_sb[:, :, r, :],
                                  hf_full[r].rearrange("d p t -> p d t"))
            for vc in range(NV):
                lmr = lmp.tile([P, DI, 512], dtb, tag="lmr")
                nc.sync.dma_start(lmr, lmhp[vc])
                for tcn in range(NCORES * TL // P):
                    r, lt = tcn // 4, tcn % 4
                    ps = pp.tile([P, 512], dt, tag="mm", bufs=4)
                    for di in range(DI):
                        nc.tensor.matmul(
                            ps,
                            hf_sb[:, di, r, lt * P:(lt + 1) * P],
                            lmr[:, di, :],
                            start=(di == 0), stop=(di == DI - 1))
                    ot = outp.tile([P, 512], dt, tag="ot")
                    nc.scalar.activation(ot, ps, AF.Copy)
                    nc.sync.dma_start(
                        logits_d[tcn * P:(tcn + 1) * P,
                                 vc * 512:(vc + 1) * 512], ot)

    nc.compile()
    return nc


def kernel(**inputs):
    from concourse.bass_utils import run_bass_kernel_spmd

    shared, per_core, flags = _pack_weights(**inputs)
    key = tuple(sorted(flags.items()))
    if key not in _CACHE:
        _CACHE[key] = _build(flags)
    nc = _CACHE[key]

    in_maps = []
    for r in range(NCORES):
        m = dict(shared)
        m.update(per_core[r])
        in_maps.append(m)
    res = run_bass_kernel_spmd(nc, in_maps, list(range(NCORES)))
    logits = np.zeros((B, S, V), dtype=np.float32)
    flat = logits.reshape(B * S, V)
    for r in range(NCORES):
        flat[:, r * VSH:(r + 1) * VSH] = res.results[r]['logits'][:, :VSH]
    return logits
